# revision 3
# baseline (speedup 1.0000x reference)
"""Trainium2 Bass kernel for AtlasMemoryPoly (dense_mlp).

Reference (DIM=256, HIDDEN=1024, POLY=33152, x:(2,1024,256)):
    x_poly = [x, x_i*x_j for i<=j]                  # (T=2048, P=33152)
    gate   = silu(x_poly @ w2.T)                    # (T, H)
    value  = x_poly @ w3.T                          # (T, H)
    out    = x + (gate*value) @ w1.T                # (T, D)

Sharding: 8 cores = 4 t-groups (512 each) x 2 h-groups (512 each).
Each core computes its (t_local, h_local) block and a partial output
(512, 256); the host sums the 2 h-group partials per t-group, adds x.

Poly features: the 33152 poly axis is PERMUTED host-side (same
permutation applied to w2/w3 rows) into PAIRS of 128-feature tiles.
Pair q covers tiles (2q, 2q+1), generated as ONE elementwise multiply
of stacked row-window tiles (partition p, j in {0,1}):
    q0: [X0;X1] copy              (linear features)
    q1: [X0;X1] * [X0;X1]         (squares)
    q2: [X0*X1 ; ZERO-PAD]        (antipodal + pad to 260 tiles)
    q(2+d), d=1..127: [X0;X1] * xt2[d:256+d]
          j=0: X0*xT[d:d+128]      -> pairs (i, i+d)
          j=1: X1*xT[128+d:256+d]  -> pairs (128+i, (128+i+d)%256)

All data is fp8e4; matmuls are DoubleRow (K=256, 2 MACs/PE/cycle).
w2/w3 are host-scaled by 256 (fp8e4 min-normal headroom); the silu
applies 1/256 via ACT scale and the value-path 1/256 is folded into
w1 host-side, so gated = silu(acc0/256) * acc1 and w1' = w1/256.

DMA layouts are zipped host-side for 2KB-contiguous partition rows
(the HW-dynamic queues sustain ~150 GB/s at 1KB packets and choke the
PE; 2KB roughly doubles that):
  wz[q, p, w, j, h]     one 256KB DMA per pair (both w2 and w3 tiles),
                        alternating Scalar / Sync queues
  zs[dd, p, s, j, t]    one 256KB DMA per superbatch of 2 diagonal
                        pairs, on the GpSimd queue
Warmup matmuls run during the DMA head to pre-warm the PE HAM clock
gate (cold = 1.2 GHz for the first ~3.4us of activity).
"""

import sys

sys.path.insert(0, "/opt/trn_rl_repo")

import numpy as np
import ml_dtypes

DIM = 256
HIDDEN = 1024
T = 2048
POLY = DIM + DIM * (DIM + 1) // 2  # 33152
NPAIR = 130  # 260 tiles of 128 (one zero pad tile)
NCORES = 8
NHG = 2
NTG = 4
HLOC = HIDDEN // NHG  # 512
TLOC = T // NTG  # 512
NHC = HLOC // 128  # 4 h-chunks
NSB = 63  # superbatches (pairs 3..128), then single pair 129
W_SCALE = 256.0
WARMUP_MM = 8  # dummy matmuls during the DMA head to pre-warm the PE clock
PSUM_OUT = False  # DMA from PSUM is not supported (dma_start asserts SBUF/DRAM)

BF16 = ml_dtypes.bfloat16
FP8 = ml_dtypes.float8_e4m3fn


def build_perm():
    """tile-row index (260*128) -> old poly row, or -1 for the pad tile.

    Tile order: [lin0, lin1, sq0, sq1, anti, PAD, then (A_d, B_d) for
    d=1..127] where A_d rows i are pairs (i, i+d) and B_d rows i are
    pairs (128+i, (128+i+d) % 256).
    """
    i = np.arange(128)

    def pairs_to_old(a, b):
        lo = np.minimum(a, b)
        hi = np.maximum(a, b)
        return DIM + lo * DIM - lo * (lo - 1) // 2 + (hi - lo)

    chunks = [
        np.arange(0, 128),                  # lin0
        np.arange(128, 256),                # lin1
        pairs_to_old(i, i),                 # sq0
        pairs_to_old(128 + i, 128 + i),     # sq1
        pairs_to_old(i, 128 + i),           # anti
        np.full(128, -1, dtype=np.int64),   # PAD
    ]
    for d in range(1, 128):
        chunks.append(pairs_to_old(i, i + d))
        j = (128 + i + d) % 256
        chunks.append(pairs_to_old(128 + i, j))
    return np.concatenate(chunks)


_NC_CACHE = None


def _build_nc():
    from concourse import bacc, tile, mybir
    from concourse.mybir import ActivationFunctionType as AF

    nc = bacc.Bacc()
    bf = mybir.dt.bfloat16
    f8 = mybir.dt.float8e4
    f32 = mybir.dt.float32
    DR = mybir.MatmulPerfMode.DoubleRow

    xxz_d = nc.dram_tensor("xxz", (128, 2, TLOC), f8, kind="ExternalInput")
    xx2z_d = nc.dram_tensor("xx2z", (128, 2, 2, TLOC), f8, kind="ExternalInput")
    zs_d = nc.dram_tensor("zs", (NSB, 128, 2, 2, TLOC), f8, kind="ExternalInput")
    zs1_d = nc.dram_tensor("zs1", (128, 2, TLOC), f8, kind="ExternalInput")
    wz_d = nc.dram_tensor("wz", (NPAIR, 128, 2, 2, HLOC), f8, kind="ExternalInput")
    w1_d = nc.dram_tensor("w1s", (HLOC, DIM), bf, kind="ExternalInput")
    out_d = nc.dram_tensor("out", (TLOC, DIM), f32, kind="ExternalOutput")

    with tile.TileContext(nc) as tc:
        with (
            tc.tile_pool(name="xpool", bufs=1) as xpool,
            tc.tile_pool(name="shift", bufs=12) as shift,
            tc.tile_pool(name="poly", bufs=12) as poly,
            tc.tile_pool(name="wts", bufs=12) as wts,
            tc.tile_pool(name="epi", bufs=1) as epi,
            tc.tile_pool(name="ostage", bufs=4) as ostage,
            tc.tile_pool(name="psum", bufs=1, space="PSUM") as psum,
        ):
            # XX = [X0; X1] stacked pair tile (128, 2*TLOC): j-major halves
            XX = xpool.tile([128, 2 * TLOC], f8, tag="XX")
            nc.sync.dma_start(XX.rearrange("p (j t) -> p j t", j=2), xxz_d[:])
            # XX2 = [XX | XX] for 2-pair superbatched multiplies
            XX2 = xpool.tile([128, 4 * TLOC], f8, tag="XX2")
            nc.sync.dma_start(
                XX2.rearrange("p (s j t) -> p s j t", s=2, j=2), xx2z_d[:]
            )

            acc = {}
            for w in (0, 1):
                for hc in range(NHC):
                    acc[(w, hc)] = psum.tile(
                        [128, TLOC], f32, tag=f"acc{w}{hc}", name=f"acc{w}{hc}"
                    )

            if WARMUP_MM:
                # PE HAM clock-gate warmup: dummy matmuls on a zeroed tile
                # while the first DMAs are in flight. q0's start=True
                # re-clears the accumulator, so results are discarded.
                wu = xpool.tile([128, 512], bf, tag="warmup")
                nc.gpsimd.memset(wu[:], 0.0)
                for i in range(WARMUP_MM):
                    nc.tensor.matmul(
                        acc[(0, 0)][:],
                        wu[:, 0:128],
                        wu[:],
                        start=True,
                        stop=True,
                        skip_group_check=True,
                    )

            def wtile(q):
                """(128, 2, 2, HLOC) tile with both w2/w3 tiles of pair q."""
                wt = wts.tile([128, 2, 2, HLOC], f8, tag="wf8", name=f"wf{q}")
                eng = nc.scalar if q % 2 == 0 else nc.sync
                eng.dma_start(wt[:], wz_d[q])
                return wt

            def consume_pair(q, pt3):
                st = q == 0
                sp = q == NPAIR - 1
                wt = wtile(q)
                for hc in range(NHC):
                    hsl = slice(hc * 128, (hc + 1) * 128)
                    for w in (0, 1):
                        nc.tensor.matmul(
                            acc[(w, hc)][:],
                            wt[:, w, :, hsl],
                            pt3[:],
                            start=st,
                            stop=sp,
                            perf_mode=DR,
                        )

            # special pairs 0..2 individually, then diagonal pairs 2-at-a-time
            XXv = XX.rearrange("p (j t) -> p j t", j=2)
            consume_pair(0, XXv)
            pt1 = poly.tile([128, 2 * TLOC], f8, tag="poly", name="pt1")
            nc.vector.tensor_mul(pt1[:], XX[:], XX[:])
            consume_pair(1, pt1.rearrange("p (j t) -> p j t", j=2))
            pt2 = poly.tile([128, 2 * TLOC], f8, tag="poly", name="pt2")
            nc.vector.tensor_mul(pt2[:, 0:TLOC], XX[:, 0:TLOC], XX[:, TLOC : 2 * TLOC])
            nc.vector.memset(pt2[:, TLOC : 2 * TLOC], 0.0)
            consume_pair(2, pt2.rearrange("p (j t) -> p j t", j=2))

            for dd in range(NSB):
                q = 3 + 2 * dd
                sw = shift.tile([128, 4 * TLOC], f8, tag="sd", name=f"sw{q}")
                nc.gpsimd.dma_start(
                    sw.rearrange("p (s j t) -> p s j t", s=2, j=2), zs_d[dd]
                )
                ps = poly.tile([128, 4 * TLOC], f8, tag="poly", name=f"ps{q}")
                nc.vector.tensor_mul(ps[:], XX2[:], sw[:])
                psv = ps.rearrange("p (s j t) -> p s j t", s=2, j=2)
                consume_pair(q, psv[:, 0])
                consume_pair(q + 1, psv[:, 1])

            # last single pair q=129 (d=127)
            sw1 = shift.tile([128, 2 * TLOC], f8, tag="sd", name="sw129")
            nc.gpsimd.dma_start(sw1.rearrange("p (j t) -> p j t", j=2), zs1_d[:])
            pt = poly.tile([128, 2 * TLOC], f8, tag="poly", name="pt129")
            nc.vector.tensor_mul(pt[:], XX[:], sw1[:])
            consume_pair(NPAIR - 1, pt.rearrange("p (j t) -> p j t", j=2))

            # epilogue per h-chunk: gated = silu(gate/256) * value_scaled
            # (the value-path 1/256 is folded into w1 host-side)
            gated = {}
            for hc in range(NHC):
                sil = epi.tile([128, TLOC], bf, tag=f"sil{hc}", name=f"sil{hc}")
                g = epi.tile([128, TLOC], bf, tag=f"gated{hc}", name=f"g{hc}")
                nc.scalar.activation(
                    sil[:], acc[(0, hc)][:], AF.Silu, scale=1.0 / W_SCALE
                )
                nc.vector.tensor_mul(g[:], sil[:], acc[(1, hc)][:])
                gated[hc] = g

            w1t = {}
            for hc in range(NHC):
                wt1 = xpool.tile([128, DIM], bf, tag=f"w1_{hc}", name=f"w1_{hc}")
                nc.gpsimd.dma_start(wt1[:], w1_d[hc * 128 : (hc + 1) * 128, :])
                w1t[hc] = wt1

            for tc_i in range(TLOC // 128):
                ops = psum.tile(
                    [128, DIM],
                    f32,
                    tag=f"acc{tc_i % 2}{(tc_i // 2) % 2}",
                    name=f"ops{tc_i}",
                )
                tsl = slice(tc_i * 128, (tc_i + 1) * 128)
                for hc in range(NHC):
                    nc.tensor.matmul(
                        ops[:],
                        gated[hc][:, tsl],
                        w1t[hc][:],
                        start=hc == 0,
                        stop=hc == NHC - 1,
                    )
                if PSUM_OUT:
                    nc.gpsimd.dma_start(out_d[tsl, :], ops[:])
                else:
                    ost = ostage.tile([128, DIM], f32, tag="ost", name=f"ost{tc_i}")
                    nc.scalar.copy(ost[:], ops[:])
                    nc.gpsimd.dma_start(out_d[tsl, :], ost[:])

    nc.finalize()
    return nc


def _get_nc():
    global _NC_CACHE
    if _NC_CACHE is None:
        _NC_CACHE = _build_nc()
    return _NC_CACHE


def prepare_inputs(x, w1, w2, w3):
    """Host-side shard prep. Returns in_maps for the 8 cores."""
    perm = build_perm()  # (260*128,) with -1 for pad rows
    xt1 = np.ascontiguousarray(x.reshape(T, DIM).T).astype(FP8)  # (256, 2048)
    xt2 = np.concatenate([xt1, xt1], axis=0)  # (512, 2048)

    def to_pairs(w):  # (HIDDEN, POLY) -> (NPAIR, 128, 2, HIDDEN) f32 scaled
        wt = w.T * W_SCALE  # (POLY, HIDDEN)
        wt = np.concatenate([wt, np.zeros((1, HIDDEN), wt.dtype)], axis=0)
        g = wt[perm]  # perm -1 -> last (zero) row
        # row layout: pair q, tile j, partition k  ->  row (2q+j)*128+k
        return g.reshape(NPAIR, 2, 128, HIDDEN).transpose(0, 2, 1, 3)

    w2p = to_pairs(w2)
    w3p = to_pairs(w3)
    # wz[q, p, w, j, h]: both weights zipped for 2KB-row DMAs
    wzip = np.stack([w2p, w3p], axis=2).astype(FP8)  # (NPAIR, 128, 2, 2, HIDDEN)
    w1t = np.ascontiguousarray(w1.T * (1.0 / W_SCALE)).astype(BF16)  # (1024, 256)

    p = np.arange(128)
    jj = np.arange(2)
    # zs row index [dd, p, s, j] -> xt2 row (2dd+1+s) + 128j + p
    dd = np.arange(NSB)
    ss = np.arange(2)
    zs_rows = (
        (2 * dd[:, None, None, None] + 1 + ss[None, None, :, None])
        + 128 * jj[None, None, None, :]
        + p[None, :, None, None]
    )  # (NSB, 128, 2, 2)
    xx_rows = 128 * jj[None, :] + p[:, None]  # (128, 2)
    zs1_rows = 127 + xx_rows

    in_maps = []
    tg_cache = {}
    for c in range(NCORES):
        tg, hg = divmod(c, NHG)
        tsl = slice(tg * TLOC, (tg + 1) * TLOC)
        hsl = slice(hg * HLOC, (hg + 1) * HLOC)
        if tg not in tg_cache:
            xt_t = xt2[:, tsl]  # (512, TLOC)
            xxz = np.ascontiguousarray(xt_t[xx_rows])  # (128, 2, TLOC)
            xx2z = np.ascontiguousarray(
                np.broadcast_to(xxz[:, None], (128, 2, 2, TLOC))
            )
            zs = np.ascontiguousarray(xt_t[zs_rows])  # (NSB, 128, 2, 2, TLOC)
            zs1 = np.ascontiguousarray(xt_t[zs1_rows])  # (128, 2, TLOC)
            tg_cache[tg] = (xxz, xx2z, zs, zs1)
        xxz, xx2z, zs, zs1 = tg_cache[tg]
        in_maps.append(
            {
                "xxz": xxz,
                "xx2z": xx2z,
                "zs": zs,
                "zs1": zs1,
                "wz": np.ascontiguousarray(wzip[:, :, :, :, hsl]),
                "w1s": np.ascontiguousarray(w1t[hsl, :]),
            }
        )
    return in_maps


def run(x, w1, w2, w3, trace=False, trace_kwargs=None):
    from concourse.bass_utils import run_bass_kernel_spmd

    nc = _get_nc()
    in_maps = prepare_inputs(x, w1, w2, w3)
    last_err = None
    for attempt in range(3):
        try:
            res = run_bass_kernel_spmd(
                nc,
                in_maps,
                core_ids=list(range(NCORES)),
                trace=trace,
                **(trace_kwargs or {}),
            )
            break
        except Exception as e:  # transient device wedge (e.g. NRT unrecoverable)
            last_err = e
            import time as _time

            _time.sleep(5)
    else:
        raise last_err
    out = np.empty((T, DIM), dtype=np.float64)
    for tg in range(NTG):
        tsl = slice(tg * TLOC, (tg + 1) * TLOC)
        accs = np.zeros((TLOC, DIM), dtype=np.float64)
        for hg in range(NHG):
            accs += res.results[tg * NHG + hg]["out"].astype(np.float64)
        out[tsl] = x.reshape(T, DIM)[tsl].astype(np.float64) + accs
    return out.astype(np.float32).reshape(x.shape), res


def kernel(x, w1, w2, w3):
    out, _ = run(np.asarray(x), np.asarray(w1), np.asarray(w2), np.asarray(w3))
    return out


# revision 8
# speedup vs baseline: 1.0336x; 1.0336x over previous
"""Trainium2 Bass kernel for AtlasMemoryPoly (dense_mlp).

Reference (DIM=256, HIDDEN=1024, POLY=33152, x:(2,1024,256)):
    x_poly = [x, x_i*x_j for i<=j]                  # (T=2048, P=33152)
    gate   = silu(x_poly @ w2.T)                    # (T, H)
    value  = x_poly @ w3.T                          # (T, H)
    out    = x + (gate*value) @ w1.T                # (T, D)

Sharding: 8 cores = 4 t-groups (512 each) x 2 h-groups (512 each).
Each core computes its (t_local, h_local) block and a partial output
(512, 256); the host sums the 2 h-group partials per t-group, adds x.

Poly features: the 33152 poly axis is PERMUTED host-side (same
permutation applied to w2/w3 rows) into PAIRS of 128-feature tiles.
Pair q covers tiles (2q, 2q+1), generated as ONE elementwise multiply
of stacked row-window tiles (partition p, j in {0,1}):
    q0: [X0;X1] copy              (linear features)
    q1: [X0;X1] * [X0;X1]         (squares)
    q2: [X0*X1 ; ZERO-PAD]        (antipodal + pad to 260 tiles)
    q(2+d), d=1..127: [X0;X1] * xt2[d:256+d]
          j=0: X0*xT[d:d+128]      -> pairs (i, i+d)
          j=1: X1*xT[128+d:256+d]  -> pairs (128+i, (128+i+d)%256)

All data is fp8e4; matmuls are DoubleRow (K=256, 2 MACs/PE/cycle).
w2/w3 are host-scaled by 256 (fp8e4 min-normal headroom); the silu
applies 1/256 via ACT scale and the value-path 1/256 is folded into
w1 host-side, so gated = silu(acc0/256) * acc1 and w1' = w1/256.

DMA layouts are zipped host-side for 2KB-contiguous partition rows.
Queue budget (each HW-dynamic queue sustains only ~130-150 GB/s):
  Scalar: even-pair wz + even-dd zs (~25 MB)
  Sync:   odd-pair wz + odd-dd zs (~25 MB) + 2 output tiles
  GpSimd: head/tail one-offs only (XX, XX2, zs1, w1, 2 outputs) - its
          software queue's final DRAIN costs ~80ns per DMA issued, so
          the bulk streams must stay off it
Warmup matmuls run during the DMA head to pre-warm the PE HAM clock
gate (cold = 1.2 GHz for the first ~3.4us of activity); the wu memset
is the first GpSimd instruction so warmup starts at ~6.6us.
"""

import sys

sys.path.insert(0, "/opt/trn_rl_repo")

import numpy as np
import ml_dtypes

DIM = 256
HIDDEN = 1024
T = 2048
POLY = DIM + DIM * (DIM + 1) // 2  # 33152
NPAIR = 130  # 260 tiles of 128 (one zero pad tile)
NCORES = 8
NHG = 2
NTG = 4
HLOC = HIDDEN // NHG  # 512
TLOC = T // NTG  # 512
NHC = HLOC // 128  # 4 h-chunks
NSB = 63  # superbatches (pairs 3..128), then single pair 129
W_SCALE = 256.0
WARMUP_MM = 9  # dummy matmuls during the DMA head to pre-warm the PE clock

BF16 = ml_dtypes.bfloat16
FP8 = ml_dtypes.float8_e4m3fn


def build_perm():
    """tile-row index (260*128) -> old poly row, or -1 for the pad tile.

    Tile order: [lin0, lin1, sq0, sq1, anti, PAD, then (A_d, B_d) for
    d=1..127] where A_d rows i are pairs (i, i+d) and B_d rows i are
    pairs (128+i, (128+i+d) % 256).
    """
    i = np.arange(128)

    def pairs_to_old(a, b):
        lo = np.minimum(a, b)
        hi = np.maximum(a, b)
        return DIM + lo * DIM - lo * (lo - 1) // 2 + (hi - lo)

    chunks = [
        np.arange(0, 128),                  # lin0
        np.arange(128, 256),                # lin1
        pairs_to_old(i, i),                 # sq0
        pairs_to_old(128 + i, 128 + i),     # sq1
        pairs_to_old(i, 128 + i),           # anti
        np.full(128, -1, dtype=np.int64),   # PAD
    ]
    for d in range(1, 128):
        chunks.append(pairs_to_old(i, i + d))
        j = (128 + i + d) % 256
        chunks.append(pairs_to_old(128 + i, j))
    return np.concatenate(chunks)


_NC_CACHE = None


def _build_nc():
    from concourse import bacc, tile, mybir
    from concourse.mybir import ActivationFunctionType as AF

    nc = bacc.Bacc()
    bf = mybir.dt.bfloat16
    f8 = mybir.dt.float8e4
    f32 = mybir.dt.float32
    DR = mybir.MatmulPerfMode.DoubleRow

    xxz_d = nc.dram_tensor("xxz", (128, 2, TLOC), f8, kind="ExternalInput")
    xx2z_d = nc.dram_tensor("xx2z", (128, 2, 2, TLOC), f8, kind="ExternalInput")
    zs_d = nc.dram_tensor("zs", (NSB, 128, 2, 2, TLOC), f8, kind="ExternalInput")
    zs1_d = nc.dram_tensor("zs1", (128, 2, TLOC), f8, kind="ExternalInput")
    wz_d = nc.dram_tensor("wz", (NPAIR, 128, 2, 2, HLOC), f8, kind="ExternalInput")
    w1_d = nc.dram_tensor("w1s", (HLOC, DIM), bf, kind="ExternalInput")
    out_d = nc.dram_tensor("out", (TLOC, DIM), f32, kind="ExternalOutput")

    with tile.TileContext(nc) as tc:
        with (
            tc.tile_pool(name="xpool", bufs=1) as xpool,
            tc.tile_pool(name="shift", bufs=14) as shift,
            tc.tile_pool(name="poly", bufs=12) as poly,
            tc.tile_pool(name="wts", bufs=14) as wts,
            tc.tile_pool(name="epi", bufs=1) as epi,
            tc.tile_pool(name="ostage", bufs=4) as ostage,
            tc.tile_pool(name="psum", bufs=1, space="PSUM") as psum,
        ):
            acc = {}
            for w in (0, 1):
                for hc in range(NHC):
                    acc[(w, hc)] = psum.tile(
                        [128, TLOC], f32, tag=f"acc{w}{hc}", name=f"acc{w}{hc}"
                    )

            wu = xpool.tile([128, 512], bf, tag="warmup")
            if WARMUP_MM:
                # First GpSimd instruction so warmup can start ASAP.
                nc.gpsimd.memset(wu[:], 0.0)

            # XX = [X0; X1] stacked pair tile (128, 2*TLOC): j-major halves
            XX = xpool.tile([128, 2 * TLOC], f8, tag="XX")
            nc.gpsimd.dma_start(XX.rearrange("p (j t) -> p j t", j=2), xxz_d[:])
            # XX2 = [XX | XX] for 2-pair superbatched multiplies
            XX2 = xpool.tile([128, 4 * TLOC], f8, tag="XX2")
            nc.gpsimd.dma_start(
                XX2.rearrange("p (s j t) -> p s j t", s=2, j=2), xx2z_d[:]
            )
            # last single pair's window + w1: resident from the start
            sw1 = xpool.tile([128, 2 * TLOC], f8, tag="sw129")
            nc.gpsimd.dma_start(sw1.rearrange("p (j t) -> p j t", j=2), zs1_d[:])
            w1t = {}
            for hc in range(NHC):
                wt1 = xpool.tile([128, DIM], bf, tag=f"w1_{hc}", name=f"w1_{hc}")
                nc.gpsimd.dma_start(wt1[:], w1_d[hc * 128 : (hc + 1) * 128, :])
                w1t[hc] = wt1

            if WARMUP_MM:
                # PE HAM clock-gate warmup: dummy matmuls on a zeroed tile
                # while the first DMAs are in flight. q0's start=True
                # re-clears the accumulator, so results are discarded.
                for i in range(WARMUP_MM):
                    nc.tensor.matmul(
                        acc[(0, 0)][:],
                        wu[:, 0:128],
                        wu[:],
                        start=True,
                        stop=True,
                        skip_group_check=True,
                    )

            def wtile(q):
                """(128, 2, 2, HLOC) tile with both w2/w3 tiles of pair q."""
                wt = wts.tile([128, 2, 2, HLOC], f8, tag="wf8", name=f"wf{q}")
                eng = nc.scalar if q % 2 == 0 else nc.sync
                eng.dma_start(wt[:], wz_d[q])
                return wt

            def consume_pair(q, pt3):
                st = q == 0
                sp = q == NPAIR - 1
                wt = wtile(q)
                for hc in range(NHC):
                    hsl = slice(hc * 128, (hc + 1) * 128)
                    for w in (0, 1):
                        nc.tensor.matmul(
                            acc[(w, hc)][:],
                            wt[:, w, :, hsl],
                            pt3[:],
                            start=st,
                            stop=sp,
                            perf_mode=DR,
                        )

            # special pairs 0..2 individually, then diagonal pairs 2-at-a-time
            XXv = XX.rearrange("p (j t) -> p j t", j=2)
            consume_pair(0, XXv)
            pt1 = poly.tile([128, 2 * TLOC], f8, tag="poly", name="pt1")
            nc.vector.tensor_mul(pt1[:], XX[:], XX[:])
            consume_pair(1, pt1.rearrange("p (j t) -> p j t", j=2))
            pt2 = poly.tile([128, 2 * TLOC], f8, tag="poly", name="pt2")
            nc.vector.tensor_mul(pt2[:, 0:TLOC], XX[:, 0:TLOC], XX[:, TLOC : 2 * TLOC])
            nc.vector.memset(pt2[:, TLOC : 2 * TLOC], 0.0)
            consume_pair(2, pt2.rearrange("p (j t) -> p j t", j=2))

            for dd in range(NSB):
                q = 3 + 2 * dd
                sw = shift.tile([128, 4 * TLOC], f8, tag="sd", name=f"sw{q}")
                eng = nc.scalar if dd % 2 == 0 else nc.sync
                eng.dma_start(
                    sw.rearrange("p (s j t) -> p s j t", s=2, j=2), zs_d[dd]
                )
                ps = poly.tile([128, 4 * TLOC], f8, tag="poly", name=f"ps{q}")
                nc.vector.tensor_mul(ps[:], XX2[:], sw[:])
                psv = ps.rearrange("p (s j t) -> p s j t", s=2, j=2)
                consume_pair(q, psv[:, 0])
                consume_pair(q + 1, psv[:, 1])

            # last single pair q=129 (d=127): window already resident (sw1)
            pt = poly.tile([128, 2 * TLOC], f8, tag="poly", name="pt129")
            nc.vector.tensor_mul(pt[:], XX[:], sw1[:])
            consume_pair(NPAIR - 1, pt.rearrange("p (j t) -> p j t", j=2))

            # epilogue per h-chunk: gated = silu(gate/256) * value_scaled
            # (the value-path 1/256 is folded into w1 host-side)
            gated = {}
            for hc in range(NHC):
                sil = epi.tile([128, TLOC], bf, tag=f"sil{hc}", name=f"sil{hc}")
                g = epi.tile([128, TLOC], bf, tag=f"gated{hc}", name=f"g{hc}")
                nc.scalar.activation(
                    sil[:], acc[(0, hc)][:], AF.Silu, scale=1.0 / W_SCALE
                )
                nc.vector.tensor_mul(g[:], sil[:], acc[(1, hc)][:])
                gated[hc] = g

            for tc_i in range(TLOC // 128):
                ops = psum.tile(
                    [128, DIM],
                    f32,
                    tag=f"acc{tc_i % 2}{(tc_i // 2) % 2}",
                    name=f"ops{tc_i}",
                )
                tsl = slice(tc_i * 128, (tc_i + 1) * 128)
                for hc in range(NHC):
                    nc.tensor.matmul(
                        ops[:],
                        gated[hc][:, tsl],
                        w1t[hc][:],
                        start=hc == 0,
                        stop=hc == NHC - 1,
                    )
                ost = ostage.tile([128, DIM], f32, tag="ost", name=f"ost{tc_i}")
                # alternate ACT / DVE so the 4 PSUM->SBUF copies run in parallel
                if tc_i % 2 == 0:
                    nc.scalar.copy(ost[:], ops[:])
                else:
                    nc.vector.tensor_scalar_mul(ost[:], ops[:], 1.0)
                # output DMAs split across the idle GpSimd and Sync queues
                eng = nc.gpsimd if tc_i % 2 == 0 else nc.sync
                eng.dma_start(out_d[tsl, :], ost[:])

    nc.finalize()
    return nc


def _get_nc():
    global _NC_CACHE
    if _NC_CACHE is None:
        _NC_CACHE = _build_nc()
    return _NC_CACHE


def prepare_inputs(x, w1, w2, w3):
    """Host-side shard prep. Returns in_maps for the 8 cores."""
    perm = build_perm()  # (260*128,) with -1 for pad rows
    xt1 = np.ascontiguousarray(x.reshape(T, DIM).T).astype(FP8)  # (256, 2048)
    xt2 = np.concatenate([xt1, xt1], axis=0)  # (512, 2048)

    def to_pairs(w):  # (HIDDEN, POLY) -> (NPAIR, 128, 2, HIDDEN) f32 scaled
        wt = w.T * W_SCALE  # (POLY, HIDDEN)
        wt = np.concatenate([wt, np.zeros((1, HIDDEN), wt.dtype)], axis=0)
        g = wt[perm]  # perm -1 -> last (zero) row
        # row layout: pair q, tile j, partition k  ->  row (2q+j)*128+k
        return g.reshape(NPAIR, 2, 128, HIDDEN).transpose(0, 2, 1, 3)

    w2p = to_pairs(w2)
    w3p = to_pairs(w3)
    # wz[q, p, w, j, h]: both weights zipped for 2KB-row DMAs
    wzip = np.stack([w2p, w3p], axis=2).astype(FP8)  # (NPAIR, 128, 2, 2, HIDDEN)
    w1t = np.ascontiguousarray(w1.T * (1.0 / W_SCALE)).astype(BF16)  # (1024, 256)

    p = np.arange(128)
    jj = np.arange(2)
    # zs row index [dd, p, s, j] -> xt2 row (2dd+1+s) + 128j + p
    dd = np.arange(NSB)
    ss = np.arange(2)
    zs_rows = (
        (2 * dd[:, None, None, None] + 1 + ss[None, None, :, None])
        + 128 * jj[None, None, None, :]
        + p[None, :, None, None]
    )  # (NSB, 128, 2, 2)
    xx_rows = 128 * jj[None, :] + p[:, None]  # (128, 2)
    zs1_rows = 127 + xx_rows

    in_maps = []
    tg_cache = {}
    for c in range(NCORES):
        tg, hg = divmod(c, NHG)
        tsl = slice(tg * TLOC, (tg + 1) * TLOC)
        hsl = slice(hg * HLOC, (hg + 1) * HLOC)
        if tg not in tg_cache:
            xt_t = xt2[:, tsl]  # (512, TLOC)
            xxz = np.ascontiguousarray(xt_t[xx_rows])  # (128, 2, TLOC)
            xx2z = np.ascontiguousarray(
                np.broadcast_to(xxz[:, None], (128, 2, 2, TLOC))
            )
            zs = np.ascontiguousarray(xt_t[zs_rows])  # (NSB, 128, 2, 2, TLOC)
            zs1 = np.ascontiguousarray(xt_t[zs1_rows])  # (128, 2, TLOC)
            tg_cache[tg] = (xxz, xx2z, zs, zs1)
        xxz, xx2z, zs, zs1 = tg_cache[tg]
        in_maps.append(
            {
                "xxz": xxz,
                "xx2z": xx2z,
                "zs": zs,
                "zs1": zs1,
                "wz": np.ascontiguousarray(wzip[:, :, :, :, hsl]),
                "w1s": np.ascontiguousarray(w1t[hsl, :]),
            }
        )
    return in_maps


def run(x, w1, w2, w3, trace=False, trace_kwargs=None):
    from concourse.bass_utils import run_bass_kernel_spmd

    nc = _get_nc()
    in_maps = prepare_inputs(x, w1, w2, w3)
    last_err = None
    for attempt in range(3):
        try:
            res = run_bass_kernel_spmd(
                nc,
                in_maps,
                core_ids=list(range(NCORES)),
                trace=trace,
                **(trace_kwargs or {}),
            )
            break
        except Exception as e:  # transient device wedge (e.g. NRT unrecoverable)
            last_err = e
            import time as _time

            _time.sleep(5)
    else:
        raise last_err
    out = np.empty((T, DIM), dtype=np.float64)
    for tg in range(NTG):
        tsl = slice(tg * TLOC, (tg + 1) * TLOC)
        accs = np.zeros((TLOC, DIM), dtype=np.float64)
        for hg in range(NHG):
            accs += res.results[tg * NHG + hg]["out"].astype(np.float64)
        out[tsl] = x.reshape(T, DIM)[tsl].astype(np.float64) + accs
    return out.astype(np.float32).reshape(x.shape), res


def kernel(x, w1, w2, w3):
    out, _ = run(np.asarray(x), np.asarray(w1), np.asarray(w2), np.asarray(w3))
    return out


# revision 12
# speedup vs baseline: 1.0394x; 1.0056x over previous
"""Trainium2 Bass kernel for AtlasMemoryPoly (dense_mlp).

Reference (DIM=256, HIDDEN=1024, POLY=33152, x:(2,1024,256)):
    x_poly = [x, x_i*x_j for i<=j]                  # (T=2048, P=33152)
    gate   = silu(x_poly @ w2.T)                    # (T, H)
    value  = x_poly @ w3.T                          # (T, H)
    out    = x + (gate*value) @ w1.T                # (T, D)

Sharding: 8 cores = 4 t-groups (512 each) x 2 h-groups (512 each).
Each core computes its (t_local, h_local) block and a partial output
(512, 256); the host sums the 2 h-group partials per t-group, adds x.

Poly features: the 33152 poly axis is PERMUTED host-side (same
permutation applied to w2/w3 rows) into PAIRS of 128-feature tiles.
Pair q covers tiles (2q, 2q+1), generated as ONE elementwise multiply
of stacked row-window tiles (partition p, j in {0,1}):
    q0: [X0;X1] copy              (linear features)
    q1: [X0;X1] * [X0;X1]         (squares)
    q2: [X0*X1 ; ZERO-PAD]        (antipodal + pad to 260 tiles)
    q(2+d), d=1..127: [X0;X1] * xt2[d:256+d]
          j=0: X0*xT[d:d+128]      -> pairs (i, i+d)
          j=1: X1*xT[128+d:256+d]  -> pairs (128+i, (128+i+d)%256)

All data is fp8e4; matmuls are DoubleRow (K=256, 2 MACs/PE/cycle).
w2/w3 are host-scaled by 256 (fp8e4 min-normal headroom); the silu
applies 1/256 via ACT scale and the value-path 1/256 is folded into
w1 host-side, so gated = silu(acc0/256) * acc1 and w1' = w1/256.

DMA layouts are zipped host-side for 2KB-contiguous partition rows.
Queue budget (each HW-dynamic queue sustains only ~130-150 GB/s):
  Scalar: even-pair wz + even-dd zs (~25 MB)
  Sync:   odd-pair wz + odd-dd zs (~25 MB) + 2 output tiles
  GpSimd: head/tail one-offs only (XX, XX2, zs1, w1, 2 outputs) - its
          software queue's final DRAIN costs ~80ns per DMA issued, so
          the bulk streams must stay off it
Warmup matmuls run during the DMA head to pre-warm the PE HAM clock
gate (cold = 1.2 GHz for the first ~3.4us of activity); the wu memset
is the first GpSimd instruction so warmup starts at ~6.6us.
"""

import sys

sys.path.insert(0, "/opt/trn_rl_repo")

import numpy as np
import ml_dtypes

DIM = 256
HIDDEN = 1024
T = 2048
POLY = DIM + DIM * (DIM + 1) // 2  # 33152
NPAIR = 130  # 260 tiles of 128 (one zero pad tile)
NCORES = 8
NHG = 2
NTG = 4
HLOC = HIDDEN // NHG  # 512
TLOC = T // NTG  # 512
NHC = HLOC // 128  # 4 h-chunks
NSB = 63  # superbatches (pairs 3..128), then single pair 129
W_SCALE = 256.0
WARMUP_MM = 20  # N=256 dummy matmuls pre-warm the PE clock during the DMA head
ZS_PREFETCH = 2  # superbatch windows DMA'd ahead of the weight stream

BF16 = ml_dtypes.bfloat16
FP8 = ml_dtypes.float8_e4m3fn


def build_perm():
    """tile-row index (260*128) -> old poly row, or -1 for the pad tile.

    Tile order: [lin0, lin1, sq0, sq1, anti, PAD, then (A_d, B_d) for
    d=1..127] where A_d rows i are pairs (i, i+d) and B_d rows i are
    pairs (128+i, (128+i+d) % 256).
    """
    i = np.arange(128)

    def pairs_to_old(a, b):
        lo = np.minimum(a, b)
        hi = np.maximum(a, b)
        return DIM + lo * DIM - lo * (lo - 1) // 2 + (hi - lo)

    chunks = [
        np.arange(0, 128),                  # lin0
        np.arange(128, 256),                # lin1
        pairs_to_old(i, i),                 # sq0
        pairs_to_old(128 + i, 128 + i),     # sq1
        pairs_to_old(i, 128 + i),           # anti
        np.full(128, -1, dtype=np.int64),   # PAD
    ]
    for d in range(1, 128):
        chunks.append(pairs_to_old(i, i + d))
        j = (128 + i + d) % 256
        chunks.append(pairs_to_old(128 + i, j))
    return np.concatenate(chunks)


_NC_CACHE = None


def _build_nc():
    from concourse import bacc, tile, mybir
    from concourse.mybir import ActivationFunctionType as AF

    nc = bacc.Bacc()
    bf = mybir.dt.bfloat16
    f8 = mybir.dt.float8e4
    f32 = mybir.dt.float32
    DR = mybir.MatmulPerfMode.DoubleRow

    xxz_d = nc.dram_tensor("xxz", (128, 2, TLOC), f8, kind="ExternalInput")
    xx2z_d = nc.dram_tensor("xx2z", (128, 2, 2, TLOC), f8, kind="ExternalInput")
    zs_d = nc.dram_tensor("zs", (NSB, 128, 2, 2, TLOC), f8, kind="ExternalInput")
    zs1_d = nc.dram_tensor("zs1", (128, 2, TLOC), f8, kind="ExternalInput")
    wz_d = nc.dram_tensor("wz", (NPAIR, 128, 2, 2, HLOC), f8, kind="ExternalInput")
    w1_d = nc.dram_tensor("w1s", (HLOC, DIM), bf, kind="ExternalInput")
    out_d = nc.dram_tensor("out", (TLOC, DIM), f32, kind="ExternalOutput")

    with tile.TileContext(nc) as tc:
        with (
            tc.tile_pool(name="xpool", bufs=1) as xpool,
            tc.tile_pool(name="shift", bufs=14) as shift,
            tc.tile_pool(name="poly", bufs=12) as poly,
            tc.tile_pool(name="wts", bufs=14) as wts,
            tc.tile_pool(name="epi", bufs=1) as epi,
            tc.tile_pool(name="ostage", bufs=4) as ostage,
            tc.tile_pool(name="psum", bufs=1, space="PSUM") as psum,
        ):
            acc = {}
            for w in (0, 1):
                for hc in range(NHC):
                    acc[(w, hc)] = psum.tile(
                        [128, TLOC], f32, tag=f"acc{w}{hc}", name=f"acc{w}{hc}"
                    )

            wu = xpool.tile([128, 256], bf, tag="warmup")
            if WARMUP_MM:
                # First GpSimd instruction so warmup can start ASAP.
                nc.gpsimd.memset(wu[:], 0.0)

            # XX = [X0; X1] stacked pair tile (128, 2*TLOC): j-major halves.
            # XX/XX2 lead the Sync HW queue (the GpSimd software queue has
            # ~3us trigger->data latency).
            XX = xpool.tile([128, 2 * TLOC], f8, tag="XX")
            nc.sync.dma_start(XX.rearrange("p (j t) -> p j t", j=2), xxz_d[:])
            # XX2 = [XX | XX] for 2-pair superbatched multiplies
            XX2 = xpool.tile([128, 4 * TLOC], f8, tag="XX2")
            nc.sync.dma_start(
                XX2.rearrange("p (s j t) -> p s j t", s=2, j=2), xx2z_d[:]
            )
            # last single pair's window + w1: resident from the start
            sw1 = xpool.tile([128, 2 * TLOC], f8, tag="sw129")
            nc.gpsimd.dma_start(sw1.rearrange("p (j t) -> p j t", j=2), zs1_d[:])
            w1t = {}
            for hc in range(NHC):
                wt1 = xpool.tile([128, DIM], bf, tag=f"w1_{hc}", name=f"w1_{hc}")
                nc.gpsimd.dma_start(wt1[:], w1_d[hc * 128 : (hc + 1) * 128, :])
                w1t[hc] = wt1

            if WARMUP_MM:
                # PE HAM clock-gate warmup: dummy matmuls on a zeroed tile
                # while the first DMAs are in flight. q0's start=True
                # re-clears the accumulator, so results are discarded.
                for i in range(WARMUP_MM):
                    nc.tensor.matmul(
                        acc[(0, 0)][:, 0:256],
                        wu[:, 0:128],
                        wu[:],
                        start=True,
                        stop=True,
                        skip_group_check=True,
                    )

            def wtile(q):
                """(128, 2, 2, HLOC) tile with both w2/w3 tiles of pair q."""
                wt = wts.tile([128, 2, 2, HLOC], f8, tag="wf8", name=f"wf{q}")
                eng = nc.scalar if q % 2 == 0 else nc.sync
                eng.dma_start(wt[:], wz_d[q])
                return wt

            def consume_pair(q, pt3):
                st = q == 0
                sp = q == NPAIR - 1
                wt = wtile(q)
                for hc in range(NHC):
                    hsl = slice(hc * 128, (hc + 1) * 128)
                    for w in (0, 1):
                        nc.tensor.matmul(
                            acc[(w, hc)][:],
                            wt[:, w, :, hsl],
                            pt3[:],
                            start=st,
                            stop=sp,
                            perf_mode=DR,
                        )

            def zs_dma(dd):
                sw = shift.tile([128, 4 * TLOC], f8, tag="sd", name=f"sw{3+2*dd}")
                eng = nc.scalar if dd % 2 == 0 else nc.sync
                eng.dma_start(
                    sw.rearrange("p (s j t) -> p s j t", s=2, j=2), zs_d[dd]
                )
                return sw

            swt = {}
            # special pairs 0..2 individually, then diagonal pairs 2-at-a-time.
            # The first superbatch windows are prefetched between the early
            # weight DMAs so the DVE poly stream never starves the PE.
            XXv = XX.rearrange("p (j t) -> p j t", j=2)
            consume_pair(0, XXv)
            if ZS_PREFETCH > 0:
                swt[0] = zs_dma(0)
            pt1 = poly.tile([128, 2 * TLOC], f8, tag="poly", name="pt1")
            nc.vector.tensor_mul(pt1[:], XX[:], XX[:])
            consume_pair(1, pt1.rearrange("p (j t) -> p j t", j=2))
            if ZS_PREFETCH > 1:
                swt[1] = zs_dma(1)
            pt2 = poly.tile([128, 2 * TLOC], f8, tag="poly", name="pt2")
            nc.vector.tensor_mul(pt2[:, 0:TLOC], XX[:, 0:TLOC], XX[:, TLOC : 2 * TLOC])
            nc.vector.memset(pt2[:, TLOC : 2 * TLOC], 0.0)
            consume_pair(2, pt2.rearrange("p (j t) -> p j t", j=2))

            for dd in range(NSB):
                q = 3 + 2 * dd
                sw = swt.pop(dd) if dd in swt else zs_dma(dd)
                ps = poly.tile([128, 4 * TLOC], f8, tag="poly", name=f"ps{q}")
                nc.vector.tensor_mul(ps[:], XX2[:], sw[:])
                psv = ps.rearrange("p (s j t) -> p s j t", s=2, j=2)
                consume_pair(q, psv[:, 0])
                consume_pair(q + 1, psv[:, 1])

            # last single pair q=129 (d=127): window already resident (sw1)
            pt = poly.tile([128, 2 * TLOC], f8, tag="poly", name="pt129")
            nc.vector.tensor_mul(pt[:], XX[:], sw1[:])
            consume_pair(NPAIR - 1, pt.rearrange("p (j t) -> p j t", j=2))

            # epilogue per h-chunk: gated = silu(gate/256) * value_scaled
            # (the value-path 1/256 is folded into w1 host-side)
            gated = {}
            for hc in range(NHC):
                sil = epi.tile([128, TLOC], bf, tag=f"sil{hc}", name=f"sil{hc}")
                g = epi.tile([128, TLOC], bf, tag=f"gated{hc}", name=f"g{hc}")
                nc.scalar.activation(
                    sil[:], acc[(0, hc)][:], AF.Silu, scale=1.0 / W_SCALE
                )
                nc.vector.tensor_mul(g[:], sil[:], acc[(1, hc)][:])
                gated[hc] = g

            for tc_i in range(TLOC // 128):
                ops = psum.tile(
                    [128, DIM],
                    f32,
                    tag=f"acc{tc_i % 2}{(tc_i // 2) % 2}",
                    name=f"ops{tc_i}",
                )
                tsl = slice(tc_i * 128, (tc_i + 1) * 128)
                for hc in range(NHC):
                    nc.tensor.matmul(
                        ops[:],
                        gated[hc][:, tsl],
                        w1t[hc][:],
                        start=hc == 0,
                        stop=hc == NHC - 1,
                    )
                ost = ostage.tile([128, DIM], f32, tag="ost", name=f"ost{tc_i}")
                # alternate ACT / DVE so the 4 PSUM->SBUF copies run in parallel
                if tc_i % 2 == 0:
                    nc.scalar.copy(ost[:], ops[:])
                else:
                    nc.vector.tensor_scalar_mul(ost[:], ops[:], 1.0)
                # output DMAs split across the idle GpSimd and Sync queues
                eng = nc.gpsimd if tc_i % 2 == 0 else nc.sync
                eng.dma_start(out_d[tsl, :], ost[:])

    nc.finalize()
    return nc


def _get_nc():
    global _NC_CACHE
    if _NC_CACHE is None:
        _NC_CACHE = _build_nc()
    return _NC_CACHE


def prepare_inputs(x, w1, w2, w3):
    """Host-side shard prep. Returns in_maps for the 8 cores."""
    perm = build_perm()  # (260*128,) with -1 for pad rows
    xt1 = np.ascontiguousarray(x.reshape(T, DIM).T).astype(FP8)  # (256, 2048)
    xt2 = np.concatenate([xt1, xt1], axis=0)  # (512, 2048)

    def to_pairs(w):  # (HIDDEN, POLY) -> (NPAIR, 128, 2, HIDDEN) f32 scaled
        wt = w.T * W_SCALE  # (POLY, HIDDEN)
        wt = np.concatenate([wt, np.zeros((1, HIDDEN), wt.dtype)], axis=0)
        g = wt[perm]  # perm -1 -> last (zero) row
        # row layout: pair q, tile j, partition k  ->  row (2q+j)*128+k
        return g.reshape(NPAIR, 2, 128, HIDDEN).transpose(0, 2, 1, 3)

    w2p = to_pairs(w2)
    w3p = to_pairs(w3)
    # wz[q, p, w, j, h]: both weights zipped for 2KB-row DMAs
    wzip = np.stack([w2p, w3p], axis=2).astype(FP8)  # (NPAIR, 128, 2, 2, HIDDEN)
    w1t = np.ascontiguousarray(w1.T * (1.0 / W_SCALE)).astype(BF16)  # (1024, 256)

    p = np.arange(128)
    jj = np.arange(2)
    # zs row index [dd, p, s, j] -> xt2 row (2dd+1+s) + 128j + p
    dd = np.arange(NSB)
    ss = np.arange(2)
    zs_rows = (
        (2 * dd[:, None, None, None] + 1 + ss[None, None, :, None])
        + 128 * jj[None, None, None, :]
        + p[None, :, None, None]
    )  # (NSB, 128, 2, 2)
    xx_rows = 128 * jj[None, :] + p[:, None]  # (128, 2)
    zs1_rows = 127 + xx_rows

    in_maps = []
    tg_cache = {}
    for c in range(NCORES):
        tg, hg = divmod(c, NHG)
        tsl = slice(tg * TLOC, (tg + 1) * TLOC)
        hsl = slice(hg * HLOC, (hg + 1) * HLOC)
        if tg not in tg_cache:
            xt_t = xt2[:, tsl]  # (512, TLOC)
            xxz = np.ascontiguousarray(xt_t[xx_rows])  # (128, 2, TLOC)
            xx2z = np.ascontiguousarray(
                np.broadcast_to(xxz[:, None], (128, 2, 2, TLOC))
            )
            zs = np.ascontiguousarray(xt_t[zs_rows])  # (NSB, 128, 2, 2, TLOC)
            zs1 = np.ascontiguousarray(xt_t[zs1_rows])  # (128, 2, TLOC)
            tg_cache[tg] = (xxz, xx2z, zs, zs1)
        xxz, xx2z, zs, zs1 = tg_cache[tg]
        in_maps.append(
            {
                "xxz": xxz,
                "xx2z": xx2z,
                "zs": zs,
                "zs1": zs1,
                "wz": np.ascontiguousarray(wzip[:, :, :, :, hsl]),
                "w1s": np.ascontiguousarray(w1t[hsl, :]),
            }
        )
    return in_maps


def run(x, w1, w2, w3, trace=False, trace_kwargs=None):
    from concourse.bass_utils import run_bass_kernel_spmd

    nc = _get_nc()
    in_maps = prepare_inputs(x, w1, w2, w3)
    last_err = None
    for attempt in range(3):
        try:
            res = run_bass_kernel_spmd(
                nc,
                in_maps,
                core_ids=list(range(NCORES)),
                trace=trace,
                **(trace_kwargs or {}),
            )
            break
        except Exception as e:  # transient device wedge (e.g. NRT unrecoverable)
            last_err = e
            import time as _time

            _time.sleep(5)
    else:
        raise last_err
    out = np.empty((T, DIM), dtype=np.float64)
    for tg in range(NTG):
        tsl = slice(tg * TLOC, (tg + 1) * TLOC)
        accs = np.zeros((TLOC, DIM), dtype=np.float64)
        for hg in range(NHG):
            accs += res.results[tg * NHG + hg]["out"].astype(np.float64)
        out[tsl] = x.reshape(T, DIM)[tsl].astype(np.float64) + accs
    return out.astype(np.float32).reshape(x.shape), res


def kernel(x, w1, w2, w3):
    out, _ = run(np.asarray(x), np.asarray(w1), np.asarray(w2), np.asarray(w3))
    return out
